# revision 19
# baseline (speedup 1.0000x reference)
"""Trainium2 kernel for nn_HSCR_67396626809127 (gnn_message_passing).

The reference network (fc1/fc2 -> 24-step KTD kinematic-tree recurrence ->
cam/pose/shape heads) contains no nonlinearity (dropout is identity in eval
mode), so the whole module is one affine map:

    out[157] = W @ [x(256) | init_pose(144) | init_shape(10) | init_cam(3)] + b

W [157,413] / b [157] are composed on host in float64 from the small weight
tensors (<5MB total), with the bias folded in as a constant-ones feature row
(K = 414).  The device runs a data-parallel matmul over the B*T = 32768
tokens; each of the 8 cores handles TPC = 4096 tokens.

Precision plan (rel tolerance is 2e-2): activations are quantized to int8
with one scale per feature column, scales folded into the weights; the
SWDGE DMA casts int8 -> bf16 inline (verified exact on HW), the PE runs
bf16 with f32 PSUM accumulation, outputs return as bf16.  Measured rel
err ~1.0e-2, and input HBM traffic halves vs bf16.

Device organization (activations-stationary):
  - stationary lhsT = activation tile [128 feats, 128 tokens] (contiguous,
    so PE fast-weight-load + background weight buffer engage),
    moving rhs = W^T k-chunk [128 feats, 157 outs]
  - psum [128 tokens, 157 outs] accumulates the 4 k-chunks (K = 414
    packed as 3x128 + 30): ~4.9 PE column-streams per token instead of 8.
  - token t of a core maps to (section s, partition p, group q) via
    t = 1024*s + 8*p + q; input DRAM is packed per-section-contiguous
    (3KB int8 DMA descriptors) and the output is stored in (p, half, s', q)
    order so each half-store writes 5KB contiguous runs per partition.
  - int8 input loads ride the gpsimd SWDGE queue (cast during DMA);
    weights/r3 ride HWDGE; output stores use the HWDGE rings, which are
    idle by then.  PSUM->SBUF copies rotate vector/scalar/gpsimd so the
    8 psum banks recycle as fast as the PE refills them.
  - a short burst of warm-up matmuls on a memset tile runs right after
    the engine preamble so the PE HAM throttle reaches full clock before
    the real matmul stream begins.
"""

import numpy as np
import ml_dtypes

ANCESTOR_INDEX = [[], [0], [0], [0], [0, 1], [0, 2], [0, 3], [0, 1, 4],
                  [0, 2, 5], [0, 3, 6], [0, 1, 4, 7], [0, 2, 5, 8],
                  [0, 3, 6, 9], [0, 3, 6, 9], [0, 3, 6, 9], [0, 3, 6, 9, 12],
                  [0, 3, 6, 9, 13], [0, 3, 6, 9, 14], [0, 3, 6, 9, 13, 16],
                  [0, 3, 6, 9, 14, 17], [0, 3, 6, 9, 13, 16, 18],
                  [0, 3, 6, 9, 14, 17, 19], [0, 3, 6, 9, 13, 16, 18, 20],
                  [0, 3, 6, 9, 14, 17, 19, 21]]
HID = 1024
NCORES = 8
B, T = 2048, 16
NTOK = B * T                 # 32768
TPC = NTOK // NCORES         # 4096 tokens per core
NOUT = 157                   # [cam 3 | pose 144 | shape 10]
KV = 414                     # 413 input features + ones row (bias)
NSEC = 4                     # sections of 1024 tokens
NGRP = 8                     # psum groups per section (token = 1024s+8p+q)
NWARM = 14                   # warm-up matmuls (N=256) before the real stream

_PROG = {}


def _compose_affine(fc1_w, fc1_b, fc2_w, fc2_b, decshape_w, decshape_b,
                    deccam_w, deccam_b, ktd_w, ktd_b):
    """Fold the whole network into out = v @ W.T + b, v = [x|pose|shape|cam]."""
    f8 = np.float64
    fc1_w, fc1_b = fc1_w.astype(f8), fc1_b.astype(f8)
    fc2_w, fc2_b = fc2_w.astype(f8), fc2_b.astype(f8)
    decshape_w, decshape_b = decshape_w.astype(f8), decshape_b.astype(f8)
    deccam_w, deccam_b = deccam_w.astype(f8), deccam_b.astype(f8)
    ktd_w, ktd_b = ktd_w.astype(f8), ktd_b.astype(f8)

    F1x, F1s = fc1_w[:, :256], fc1_w[:, 256:266]
    F2x, F2p = fc2_w[:, :256], fc2_w[:, 256:400]

    # KTD recurrence -> pose_out = G @ xc_pose + H @ init_pose + c
    G = np.zeros((24, 6, HID)); H = np.zeros((24, 6, 144)); c = np.zeros((24, 6))
    for j, anc in enumerate(ANCESTOR_INDEX):
        Wj = ktd_w[j]
        G[j] = Wj[:, :HID]
        off = HID
        for i in anc:
            A = Wj[:, off:off + 6]; off += 6
            G[j] += A @ G[i]
            H[j] += A @ H[i]
            c[j] += A @ c[i]
        # reference concatenates init_pose[..., j:j+6] (overlapping slice)
        H[j][:, j:j + 6] += Wj[:, off:off + 6]
        c[j] += ktd_b[j]
    G = G.reshape(144, HID); H = H.reshape(144, 144); c = c.reshape(144)

    Dp, Ds, Dc = deccam_w[:, :HID], deccam_w[:, HID:2 * HID], deccam_w[:, 2 * HID:]

    W = np.zeros((NOUT, 413)); b = np.zeros(NOUT)
    W[0:3, 0:256] = Dp @ F2x + Ds @ F1x
    W[0:3, 256:400] = Dp @ F2p
    W[0:3, 400:410] = Ds @ F1s
    W[0:3, 410:413] = Dc + np.eye(3)
    b[0:3] = Dp @ fc2_b + Ds @ fc1_b + deccam_b

    W[3:147, 0:256] = G @ F2x
    W[3:147, 256:400] = G @ F2p + H + np.eye(144)
    b[3:147] = G @ fc2_b + c

    W[147:157, 0:256] = decshape_w @ F1x
    W[147:157, 400:410] = decshape_w @ F1s + np.eye(10)
    b[147:157] = decshape_w @ fc1_b + decshape_b
    return W.astype(np.float64), b.astype(np.float64)


def _build_program():
    import concourse.bass as bass
    import concourse.tile as tile
    from concourse import bacc, mybir

    f32 = mybir.dt.float32
    bf16 = mybir.dt.bfloat16
    i8 = mybir.dt.int8
    nc = bacc.Bacc("TRN2", target_bir_lowering=False, debug=False,
                   num_devices=NCORES)
    # activations packed per section: vtp[s, f, c, q, p] = feature (c*128+f)
    # of token (1024s + 8p + q), quantized int8 (per-feature scales folded
    # into wt).  lhsT slices [:, k, q, :] are contiguous.
    vtp = nc.declare_dram_parameter("vtp", [NSEC, 128, 3, NGRP, 128], i8,
                                    isOutput=False)
    vt3p = nc.declare_dram_parameter("vt3p", [NSEC, 30, NGRP, 128], bf16,
                                     isOutput=False)

    # W^T packed [128, 4, NOUT]; chunk 3 rows 30..127 are zero (unused)
    wt = nc.declare_dram_parameter("wt", [128, 4, NOUT], bf16, isOutput=False)
    # output in (p, half, s', q, o) order; host un-permutes
    ot = nc.declare_dram_parameter("ot", [128, 2, 2, NGRP, NOUT], bf16,
                                   isOutput=True)

    with tile.TileContext(nc) as tc:
        with (
            tc.tile_pool(name="wpool", bufs=1) as wpool,
            tc.tile_pool(name="rin", bufs=3) as rpool,
            tc.tile_pool(name="outp", bufs=3) as opool,
            tc.tile_pool(name="psum", bufs=1, space=bass.MemorySpace.PSUM) as ppool,
        ):
            # PE warm-up: memset a zeros tile, run a few N=512 matmuls into
            # the ps0 slot so the HAM throttle sees sustained PE activity
            # while the first input DMAs are still in flight.
            z = wpool.tile([128, 512], bf16, tag="z", name="z")
            nc.vector.memset(z[:], 0.0)
            psw = ppool.tile([128, 512], f32, tag="ps0", name="ps_warm")
            for i in range(NWARM):
                nc.tensor.matmul(psw[:, 0:256], z[:, 0:128], z[:, 0:256],
                                 start=(i == 0), stop=(i == NWARM - 1))
            zsink = wpool.tile([128, 512], bf16, tag="zsink", name="zsink")
            nc.vector.tensor_copy(zsink[:], psw[:])

            w = wpool.tile([128, 4, NOUT], bf16, tag="w", name="w")
            nc.scalar.dma_start(w[:], wt[:])

            # int8 input loads ride the gpsimd SWDGE queue (cast to bf16
            # during DMA); section 0 split per-chunk for an early PE start
            r012s, r3s = [], []
            r012_0 = rpool.tile([128, 3, NGRP, 128], bf16, tag="r012",
                                name="r012_0")
            nc.gpsimd.dma_start(r012_0[:], vtp[0])
            r3_0 = rpool.tile([30, NGRP, 128], bf16, tag="r3", name="r3_0")
            nc.sync.dma_start(r3_0[:], vt3p[0])
            r012s.append(r012_0); r3s.append(r3_0)

            def load_section(s):
                r012 = rpool.tile([128, 3, NGRP, 128], bf16, tag="r012",
                                  name=f"r012_{s}")
                nc.gpsimd.dma_start(r012[:], vtp[s])
                r3 = rpool.tile([30, NGRP, 128], bf16, tag="r3", name=f"r3_{s}")
                ring = nc.sync if s % 2 == 1 else nc.scalar
                ring.dma_start(r3[:], vt3p[s])
                r012s.append(r012)
                r3s.append(r3)

            load_section(1)

            cpeng = [None, None, None]

            for s in range(NSEC):
                if s + 1 < NSEC and s > 0:
                    load_section(s + 1)
                r012, r3 = r012s[s], r3s[s]
                pss = []
                for q in range(NGRP):
                    ps = ppool.tile([128, 512], f32, tag=f"ps{q}",
                                    name=f"ps_{s}_{q}")
                    pss.append(ps)
                for k in range(4):
                    rhs = w[:, k, :] if k < 3 else w[0:30, 3, :]
                    for q in range(NGRP):
                        lhsT = r012[:, k, q, :] if k < 3 else r3[:, q, :]
                        nc.tensor.matmul(pss[q][:, 0:NOUT], lhsT, rhs,
                                         start=(k == 0), stop=(k == 3))
                h, sh = divmod(s, 2)
                outt = opool.tile([128, NGRP, NOUT], bf16, tag="out",
                                  name=f"out_{s}")
                for q in range(NGRP):
                    if q % 2 == 0:
                        nc.vector.tensor_copy(outt[:, q, :],
                                              pss[q][:, 0:NOUT])
                    else:
                        nc.scalar.copy(outt[:, q, :], pss[q][:, 0:NOUT])
                if s == NSEC - 1:
                    nc.sync.dma_start(ot[:, h, sh, 0:4], outt[:, 0:4])
                    nc.scalar.dma_start(ot[:, h, sh, 4:8], outt[:, 4:8])
                else:
                    ring = nc.sync if s % 2 == 0 else nc.scalar
                    ring.dma_start(ot[:, h, sh], outt[:])
    nc.compile()
    return nc


def _get_program():
    if "nc" not in _PROG:
        _PROG["nc"] = _build_program()
    return _PROG["nc"]


def _make_in_maps(x, init_pose, init_shape, init_cam, fc1_w, fc1_b, fc2_w,
                  fc2_b, decshape_w, decshape_b, deccam_w, deccam_b, ktd_w,
                  ktd_b):
    bf = ml_dtypes.bfloat16
    x = np.asarray(x, dtype=np.float32)
    init_pose = np.asarray(init_pose, dtype=np.float32)
    init_shape = np.asarray(init_shape, dtype=np.float32)
    init_cam = np.asarray(init_cam, dtype=np.float32)

    W, b = _compose_affine(
        np.asarray(fc1_w), np.asarray(fc1_b), np.asarray(fc2_w),
        np.asarray(fc2_b), np.asarray(decshape_w), np.asarray(decshape_b),
        np.asarray(deccam_w), np.asarray(deccam_b), np.asarray(ktd_w),
        np.asarray(ktd_b))

    # full feature-major activation matrix [414, NTOK]
    v = np.empty((KV, NTOK), np.float32)
    v[0:256] = x.reshape(NTOK, 256).T
    v[256:400] = init_pose.reshape(NTOK, 144).T
    v[400:410] = init_shape.reshape(NTOK, 10).T
    v[410:413] = init_cam.reshape(NTOK, 3).T
    v[413] = 1.0

    # per-feature int8 quantization for features 0..383 (x + pose head);
    # scales folded into the weights.  Features 384..413 stay bf16 raw.
    scale = np.abs(v[0:384]).max(axis=1) / 127.0            # [384]
    q = np.clip(np.round(v[0:384] / scale[:, None]), -127, 127).astype(np.int8)

    W_aug = np.concatenate([W, b[:, None]], axis=1)         # [157, 414] f64
    wtk = W_aug.T.copy()                                    # [414, 157] f64
    wtk[0:384] *= scale[:, None]
    wtk = wtk.astype(np.float32).astype(bf)
    wt = np.zeros((128, 4, NOUT), bf)
    wt[:, 0:3, :] = wtk[0:384].reshape(3, 128, NOUT).transpose(1, 0, 2)
    wt[0:30, 3, :] = wtk[384:414]
    wt = np.ascontiguousarray(wt)

    in_maps = []
    for i in range(NCORES):
        qc = q[:, i * TPC:(i + 1) * TPC]                    # [384, TPC] int8
        # vtp[s, f, c, q, p] = v[c*128+f, 1024s+8p+q]
        vtp = qc.reshape(3, 128, NSEC, 128, NGRP).transpose(2, 1, 0, 4, 3)
        # chunk-3 features stay bf16 (tiny); they skip quantization
        v3 = v[384:414, i * TPC:(i + 1) * TPC].astype(bf)
        vt3p = v3.reshape(30, NSEC, 128, NGRP).transpose(1, 0, 3, 2)
        in_maps.append({
            "vtp": np.ascontiguousarray(vtp),
            "vt3p": np.ascontiguousarray(vt3p),
            "wt": wt,
        })
    return in_maps


def _assemble(results):
    out = np.empty((NTOK, NOUT), np.float32)
    for i in range(NCORES):
        # ot[p, h, s', q, o] -> token 1024*(2h+s') + 8p + q
        o = results[i]["ot"].astype(np.float32).reshape(128, NSEC, NGRP, NOUT)
        out[i * TPC:(i + 1) * TPC] = (
            o.transpose(1, 0, 2, 3).reshape(TPC, NOUT))
    return out


def kernel(x, init_pose, init_shape, init_cam, fc1_w, fc1_b, fc2_w, fc2_b,
           decshape_w, decshape_b, deccam_w, deccam_b, ktd_w, ktd_b):
    from concourse.bass_utils import run_bass_kernel_spmd

    in_maps = _make_in_maps(x, init_pose, init_shape, init_cam, fc1_w, fc1_b,
                            fc2_w, fc2_b, decshape_w, decshape_b, deccam_w,
                            deccam_b, ktd_w, ktd_b)
    nc = _get_program()
    res = run_bass_kernel_spmd(nc, in_maps, list(range(NCORES)))
    return _assemble(res.results)


# revision 20
# speedup vs baseline: 1.1463x; 1.1463x over previous
"""Trainium2 kernel for nn_HSCR_67396626809127 (gnn_message_passing).

The reference network (fc1/fc2 -> 24-step KTD kinematic-tree recurrence ->
cam/pose/shape heads) contains no nonlinearity (dropout is identity in eval
mode), so the whole module is one affine map:

    out[157] = W @ [x(256) | init_pose(144) | init_shape(10) | init_cam(3)] + b

W [157,413] / b [157] are composed on host in float64 from the small weight
tensors (<5MB total), with the bias folded in as a constant-ones feature row
(K = 414).  The device runs a data-parallel matmul over the B*T = 32768
tokens; each of the 8 cores handles TPC = 4096 tokens.

Precision plan (rel tolerance is 2e-2): activations are quantized to int8
with one scale per feature column, scales folded into the weights; the
SWDGE DMA casts int8 -> bf16 inline (verified exact on HW), the PE runs
bf16 with f32 PSUM accumulation, outputs return as bf16.  Measured rel
err ~1.0e-2, and input HBM traffic halves vs bf16.

Device organization (activations-stationary):
  - stationary lhsT = activation tile [128 feats, 128 tokens] (contiguous,
    so PE fast-weight-load + background weight buffer engage),
    moving rhs = W^T k-chunk [128 feats, 157 outs]
  - psum [128 tokens, 157 outs] accumulates the 4 k-chunks (K = 414
    packed as 3x128 + 30): ~4.9 PE column-streams per token instead of 8.
  - token t of a core maps to (section s, partition p, group q) via
    t = 1024*s + 8*p + q; input DRAM is packed per-section-contiguous
    (3KB int8 DMA descriptors) and the output is stored in (p, half, s', q)
    order so each half-store writes 5KB contiguous runs per partition.
  - int8 input loads ride the gpsimd SWDGE queue (cast during DMA);
    weights/r3 ride HWDGE; output stores use the HWDGE rings, which are
    idle by then.  PSUM->SBUF copies rotate vector/scalar/gpsimd so the
    8 psum banks recycle as fast as the PE refills them.
  - a short burst of warm-up matmuls on a memset tile runs right after
    the engine preamble so the PE HAM throttle reaches full clock before
    the real matmul stream begins.
"""

import numpy as np
import ml_dtypes

ANCESTOR_INDEX = [[], [0], [0], [0], [0, 1], [0, 2], [0, 3], [0, 1, 4],
                  [0, 2, 5], [0, 3, 6], [0, 1, 4, 7], [0, 2, 5, 8],
                  [0, 3, 6, 9], [0, 3, 6, 9], [0, 3, 6, 9], [0, 3, 6, 9, 12],
                  [0, 3, 6, 9, 13], [0, 3, 6, 9, 14], [0, 3, 6, 9, 13, 16],
                  [0, 3, 6, 9, 14, 17], [0, 3, 6, 9, 13, 16, 18],
                  [0, 3, 6, 9, 14, 17, 19], [0, 3, 6, 9, 13, 16, 18, 20],
                  [0, 3, 6, 9, 14, 17, 19, 21]]
HID = 1024
NCORES = 8
B, T = 2048, 16
NTOK = B * T                 # 32768
TPC = NTOK // NCORES         # 4096 tokens per core
NOUT = 157                   # [cam 3 | pose 144 | shape 10]
KV = 414                     # 413 input features + ones row (bias)
NSEC = 4                     # sections of 1024 tokens
NGRP = 8                     # psum groups per section (token = 1024s+8p+q)
NWARM = 20                   # warm-up matmuls (N=256) before the real stream

_PROG = {}


def _compose_affine(fc1_w, fc1_b, fc2_w, fc2_b, decshape_w, decshape_b,
                    deccam_w, deccam_b, ktd_w, ktd_b):
    """Fold the whole network into out = v @ W.T + b, v = [x|pose|shape|cam]."""
    f8 = np.float64
    fc1_w, fc1_b = fc1_w.astype(f8), fc1_b.astype(f8)
    fc2_w, fc2_b = fc2_w.astype(f8), fc2_b.astype(f8)
    decshape_w, decshape_b = decshape_w.astype(f8), decshape_b.astype(f8)
    deccam_w, deccam_b = deccam_w.astype(f8), deccam_b.astype(f8)
    ktd_w, ktd_b = ktd_w.astype(f8), ktd_b.astype(f8)

    F1x, F1s = fc1_w[:, :256], fc1_w[:, 256:266]
    F2x, F2p = fc2_w[:, :256], fc2_w[:, 256:400]

    # KTD recurrence -> pose_out = G @ xc_pose + H @ init_pose + c
    G = np.zeros((24, 6, HID)); H = np.zeros((24, 6, 144)); c = np.zeros((24, 6))
    for j, anc in enumerate(ANCESTOR_INDEX):
        Wj = ktd_w[j]
        G[j] = Wj[:, :HID]
        off = HID
        for i in anc:
            A = Wj[:, off:off + 6]; off += 6
            G[j] += A @ G[i]
            H[j] += A @ H[i]
            c[j] += A @ c[i]
        # reference concatenates init_pose[..., j:j+6] (overlapping slice)
        H[j][:, j:j + 6] += Wj[:, off:off + 6]
        c[j] += ktd_b[j]
    G = G.reshape(144, HID); H = H.reshape(144, 144); c = c.reshape(144)

    Dp, Ds, Dc = deccam_w[:, :HID], deccam_w[:, HID:2 * HID], deccam_w[:, 2 * HID:]

    W = np.zeros((NOUT, 413)); b = np.zeros(NOUT)
    W[0:3, 0:256] = Dp @ F2x + Ds @ F1x
    W[0:3, 256:400] = Dp @ F2p
    W[0:3, 400:410] = Ds @ F1s
    W[0:3, 410:413] = Dc + np.eye(3)
    b[0:3] = Dp @ fc2_b + Ds @ fc1_b + deccam_b

    W[3:147, 0:256] = G @ F2x
    W[3:147, 256:400] = G @ F2p + H + np.eye(144)
    b[3:147] = G @ fc2_b + c

    W[147:157, 0:256] = decshape_w @ F1x
    W[147:157, 400:410] = decshape_w @ F1s + np.eye(10)
    b[147:157] = decshape_w @ fc1_b + decshape_b
    return W.astype(np.float64), b.astype(np.float64)


def _build_program():
    import concourse.bass as bass
    import concourse.tile as tile
    from concourse import bacc, mybir

    f32 = mybir.dt.float32
    bf16 = mybir.dt.bfloat16
    i8 = mybir.dt.int8
    nc = bacc.Bacc("TRN2", target_bir_lowering=False, debug=False,
                   num_devices=NCORES)
    # activations packed per section: vtp[s, f, c, q, p] = feature (c*128+f)
    # of token (1024s + 8p + q), quantized int8 (per-feature scales folded
    # into wt).  lhsT slices [:, k, q, :] are contiguous.
    vtp = nc.declare_dram_parameter("vtp", [NSEC, 128, 3, NGRP, 128], i8,
                                    isOutput=False)
    vt3p = nc.declare_dram_parameter("vt3p", [NSEC, 30, NGRP, 128], bf16,
                                     isOutput=False)

    # W^T packed [128, 4, NOUT]; chunk 3 rows 30..127 are zero (unused)
    wt = nc.declare_dram_parameter("wt", [128, 4, NOUT], bf16, isOutput=False)
    # output in (p, half, s', q, o) order; host un-permutes
    ot = nc.declare_dram_parameter("ot", [128, 2, 2, NGRP, NOUT], bf16,
                                   isOutput=True)

    with tile.TileContext(nc) as tc:
        with (
            tc.tile_pool(name="wpool", bufs=1) as wpool,
            tc.tile_pool(name="rin", bufs=3) as rpool,
            tc.tile_pool(name="outp", bufs=3) as opool,
            tc.tile_pool(name="psum", bufs=1, space=bass.MemorySpace.PSUM) as ppool,
        ):
            # PE warm-up: memset a zeros tile, run a few N=512 matmuls into
            # the ps0 slot so the HAM throttle sees sustained PE activity
            # while the first input DMAs are still in flight.
            z = wpool.tile([128, 512], bf16, tag="z", name="z")
            nc.vector.memset(z[:], 0.0)
            psw = ppool.tile([128, 512], f32, tag="ps0", name="ps_warm")
            for i in range(NWARM):
                nc.tensor.matmul(psw[:, 0:256], z[:, 0:128], z[:, 0:256],
                                 start=(i == 0), stop=(i == NWARM - 1))
            zsink = wpool.tile([128, 512], bf16, tag="zsink", name="zsink")
            nc.vector.tensor_copy(zsink[:], psw[:])

            w = wpool.tile([128, 4, NOUT], bf16, tag="w", name="w")
            nc.scalar.dma_start(w[:], wt[:])

            # int8 input loads ride the gpsimd SWDGE queue (cast to bf16
            # during DMA); section 0 split per-chunk for an early PE start
            r012s, r3s = [], []
            r012_0 = rpool.tile([128, 3, NGRP, 128], bf16, tag="r012",
                                name="r012_0")
            nc.gpsimd.dma_start(r012_0[:], vtp[0])
            r3_0 = rpool.tile([30, NGRP, 128], bf16, tag="r3", name="r3_0")
            nc.sync.dma_start(r3_0[:], vt3p[0])
            r012s.append(r012_0); r3s.append(r3_0)

            def load_section(s):
                r012 = rpool.tile([128, 3, NGRP, 128], bf16, tag="r012",
                                  name=f"r012_{s}")
                nc.gpsimd.dma_start(r012[:], vtp[s])
                r3 = rpool.tile([30, NGRP, 128], bf16, tag="r3", name=f"r3_{s}")
                ring = nc.sync if s % 2 == 1 else nc.scalar
                ring.dma_start(r3[:], vt3p[s])
                r012s.append(r012)
                r3s.append(r3)

            load_section(1)

            cpeng = [None, None, None]

            for s in range(NSEC):
                if s + 1 < NSEC and s > 0:
                    load_section(s + 1)
                r012, r3 = r012s[s], r3s[s]
                pss = []
                for q in range(NGRP):
                    ps = ppool.tile([128, 512], f32, tag=f"ps{q}",
                                    name=f"ps_{s}_{q}")
                    pss.append(ps)
                for k in range(4):
                    rhs = w[:, k, :] if k < 3 else w[0:30, 3, :]
                    for q in range(NGRP):
                        lhsT = r012[:, k, q, :] if k < 3 else r3[:, q, :]
                        nc.tensor.matmul(pss[q][:, 0:NOUT], lhsT, rhs,
                                         start=(k == 0), stop=(k == 3))
                h, sh = divmod(s, 2)
                outt = opool.tile([128, NGRP, NOUT], bf16, tag="out",
                                  name=f"out_{s}")
                for q in range(NGRP):
                    if q % 2 == 0:
                        nc.vector.tensor_copy(outt[:, q, :],
                                              pss[q][:, 0:NOUT])
                    else:
                        nc.scalar.copy(outt[:, q, :], pss[q][:, 0:NOUT])
                if s == NSEC - 1:
                    nc.sync.dma_start(ot[:, h, sh, 0:4], outt[:, 0:4])
                    nc.scalar.dma_start(ot[:, h, sh, 4:8], outt[:, 4:8])
                else:
                    ring = nc.sync if s % 2 == 0 else nc.scalar
                    ring.dma_start(ot[:, h, sh], outt[:])
    nc.compile()
    return nc


def _get_program():
    if "nc" not in _PROG:
        _PROG["nc"] = _build_program()
    return _PROG["nc"]


def _make_in_maps(x, init_pose, init_shape, init_cam, fc1_w, fc1_b, fc2_w,
                  fc2_b, decshape_w, decshape_b, deccam_w, deccam_b, ktd_w,
                  ktd_b):
    bf = ml_dtypes.bfloat16
    x = np.asarray(x, dtype=np.float32)
    init_pose = np.asarray(init_pose, dtype=np.float32)
    init_shape = np.asarray(init_shape, dtype=np.float32)
    init_cam = np.asarray(init_cam, dtype=np.float32)

    W, b = _compose_affine(
        np.asarray(fc1_w), np.asarray(fc1_b), np.asarray(fc2_w),
        np.asarray(fc2_b), np.asarray(decshape_w), np.asarray(decshape_b),
        np.asarray(deccam_w), np.asarray(deccam_b), np.asarray(ktd_w),
        np.asarray(ktd_b))

    # full feature-major activation matrix [414, NTOK]
    v = np.empty((KV, NTOK), np.float32)
    v[0:256] = x.reshape(NTOK, 256).T
    v[256:400] = init_pose.reshape(NTOK, 144).T
    v[400:410] = init_shape.reshape(NTOK, 10).T
    v[410:413] = init_cam.reshape(NTOK, 3).T
    v[413] = 1.0

    # per-feature int8 quantization for features 0..383 (x + pose head);
    # scales folded into the weights.  Features 384..413 stay bf16 raw.
    scale = np.abs(v[0:384]).max(axis=1) / 127.0            # [384]
    q = np.clip(np.round(v[0:384] / scale[:, None]), -127, 127).astype(np.int8)

    W_aug = np.concatenate([W, b[:, None]], axis=1)         # [157, 414] f64
    wtk = W_aug.T.copy()                                    # [414, 157] f64
    wtk[0:384] *= scale[:, None]
    wtk = wtk.astype(np.float32).astype(bf)
    wt = np.zeros((128, 4, NOUT), bf)
    wt[:, 0:3, :] = wtk[0:384].reshape(3, 128, NOUT).transpose(1, 0, 2)
    wt[0:30, 3, :] = wtk[384:414]
    wt = np.ascontiguousarray(wt)

    in_maps = []
    for i in range(NCORES):
        qc = q[:, i * TPC:(i + 1) * TPC]                    # [384, TPC] int8
        # vtp[s, f, c, q, p] = v[c*128+f, 1024s+8p+q]
        vtp = qc.reshape(3, 128, NSEC, 128, NGRP).transpose(2, 1, 0, 4, 3)
        # chunk-3 features stay bf16 (tiny); they skip quantization
        v3 = v[384:414, i * TPC:(i + 1) * TPC].astype(bf)
        vt3p = v3.reshape(30, NSEC, 128, NGRP).transpose(1, 0, 3, 2)
        in_maps.append({
            "vtp": np.ascontiguousarray(vtp),
            "vt3p": np.ascontiguousarray(vt3p),
            "wt": wt,
        })
    return in_maps


def _assemble(results):
    out = np.empty((NTOK, NOUT), np.float32)
    for i in range(NCORES):
        # ot[p, h, s', q, o] -> token 1024*(2h+s') + 8p + q
        o = results[i]["ot"].astype(np.float32).reshape(128, NSEC, NGRP, NOUT)
        out[i * TPC:(i + 1) * TPC] = (
            o.transpose(1, 0, 2, 3).reshape(TPC, NOUT))
    return out


def kernel(x, init_pose, init_shape, init_cam, fc1_w, fc1_b, fc2_w, fc2_b,
           decshape_w, decshape_b, deccam_w, deccam_b, ktd_w, ktd_b):
    from concourse.bass_utils import run_bass_kernel_spmd

    in_maps = _make_in_maps(x, init_pose, init_shape, init_cam, fc1_w, fc1_b,
                            fc2_w, fc2_b, decshape_w, decshape_b, deccam_w,
                            deccam_b, ktd_w, ktd_b)
    nc = _get_program()
    res = run_bass_kernel_spmd(nc, in_maps, list(range(NCORES)))
    return _assemble(res.results)
